# revision 1
# baseline (speedup 1.0000x reference)
"""MoE adapter kernel for 8 Trainium2 NeuronCores.

Math (faithful to the reference): every token routes to its top-2 of 8
experts (gate = 2-layer MLP on the concat embedding); the output is the
softmax-weighted sum of the two selected experts' MLP outputs.  The
reference computes ALL experts densely and combines with weights that are
exactly zero for unselected experts, so sparse top-2 computation is
mathematically identical (4x fewer FLOPs).

Strategy:
  - Host: gate + top-2 routing in float64 (selection margins are ~5e-5,
    fp noise ~1e-6, so selection matches the fp32 reference), group the
    16384 (token, expert) pairs by expert, pad each expert's run to
    512-token blocks (always <= 39 blocks), and hand 5 blocks to each of
    the 8 cores.  Perfectly balanced by construction.
  - Device (SPMD, same program on all 8 cores; per-core weights/tokens
    arrive as input data): per 512-token block, a 2-layer MLP
    [512,5120]x[5120,4096] -> relu -> x[4096,2048], fp16 operands with
    fp32 PSUM accumulation, weights stationary / activations moving.
  - Host: scatter-add  w * (y + b2)  into the [8192, 2048] output.
"""

import os
import numpy as np

B = 8192
IN_DIM = 5120
HID = 4096
OUT_DIM = 2048
E = 8
NCORES = 8
KT1 = IN_DIM // 128          # 40 k-tiles, layer 1
HT = HID // 128              # 32 hid tiles
KT2 = HID // 128             # 32 k-tiles, layer 2
OT = OUT_DIM // 128          # 16 out tiles

LAST_RESULT = None           # BassKernelResults of the most recent run


def _build_bass(BLK, S):
    import concourse.bass as bass
    import concourse.mybir as mybir
    import concourse.tile as tile
    from concourse import bacc
    from concourse.bass import ts

    f16 = mybir.dt.float16
    f32 = mybir.dt.float32

    nc = bacc.Bacc("TRN2", target_bir_lowering=False, debug=False,
                   num_devices=NCORES)

    xt_d, w1_d, w2_d, b1_d, yt_d = [], [], [], [], []
    for s in range(S):
        xt_d.append(nc.dram_tensor(f"xt_{s}", [128, KT1 * BLK], f16,
                                   kind="ExternalInput"))
        w1_d.append(nc.dram_tensor(f"w1_{s}", [HT, 128, KT1 * 128], f16,
                                   kind="ExternalInput"))
        w2_d.append(nc.dram_tensor(f"w2_{s}", [OT, 128, KT2 * 128], f16,
                                   kind="ExternalInput"))
        b1_d.append(nc.dram_tensor(f"b1_{s}", [128, HT], f32,
                                   kind="ExternalInput"))
        yt_d.append(nc.dram_tensor(f"yt_{s}", [OT, 128, BLK], f32,
                                   kind="ExternalOutput"))

    relu = mybir.ActivationFunctionType.Relu

    wbufs = 3 if BLK <= 460 else 2       # SBUF headroom shrinks with BLK
    with tile.TileContext(nc) as tc:
        with (
            tc.tile_pool(name="xt", bufs=2) as xt_pool,
            tc.tile_pool(name="w1", bufs=wbufs) as w1_pool,
            tc.tile_pool(name="w2", bufs=wbufs) as w2_pool,
            tc.tile_pool(name="h", bufs=2) as h_pool,
            tc.tile_pool(name="b", bufs=2) as b_pool,
            tc.tile_pool(name="y", bufs=4) as y_pool,
            tc.tile_pool(name="ps1", bufs=2, space="PSUM") as ps1_pool,
            tc.tile_pool(name="ps2", bufs=2, space="PSUM") as ps2_pool,
        ):
            for s in range(S):
                xt = xt_pool.tile([128, KT1 * BLK], f16, tag="xt")
                b1t = b_pool.tile([128, HT], f32, tag="b1")
                nc.sync.dma_start(out=b1t[:], in_=b1_d[s].ap())

                h_sb = h_pool.tile([128, HT * BLK], f16, tag="h")
                for h in range(HT):
                    w1t = w1_pool.tile([128, KT1 * 128], f16, tag="w1")
                    nc.sync.dma_start(out=w1t[:], in_=w1_d[s].ap()[h])
                    ps = ps1_pool.tile([128, BLK], f32, tag="ps1")
                    for k in range(KT1):
                        if h == 0 and k % 10 == 0:
                            # xt arrives in chunks so the first matmul only
                            # gates on 1/4 of the slot's activations
                            cols = slice(k * BLK, (k + 10) * BLK)
                            nc.sync.dma_start(out=xt[:, cols],
                                              in_=xt_d[s].ap()[:, cols])
                        nc.tensor.matmul(ps[:], w1t[:, ts(k, 128)],
                                         xt[:, ts(k, BLK)],
                                         start=(k == 0), stop=(k == KT1 - 1))
                    # hT[h] = relu(psum + b1), cast to fp16
                    nc.scalar.activation(h_sb[:, ts(h, BLK)], ps[:], relu,
                                         bias=b1t[:, h:h + 1])

                for o in range(OT):
                    w2t = w2_pool.tile([128, KT2 * 128], f16, tag="w2")
                    nc.sync.dma_start(out=w2t[:], in_=w2_d[s].ap()[o])
                    ps2 = ps2_pool.tile([128, BLK], f32, tag="ps2")
                    for k in range(KT2):
                        nc.tensor.matmul(ps2[:], w2t[:, ts(k, 128)],
                                         h_sb[:, ts(k, BLK)],
                                         start=(k == 0), stop=(k == KT2 - 1))
                    yt_sb = y_pool.tile([128, BLK], f32, tag="y")
                    nc.vector.tensor_copy(yt_sb[:], ps2[:])
                    nc.sync.dma_start(out=yt_d[s].ap()[o], in_=yt_sb[:])

    nc.compile()
    return nc


_NC = {}


def _get_nc(blk, s):
    if (blk, s) not in _NC:
        _NC[(blk, s)] = _build_bass(blk, s)
    return _NC[(blk, s)]


def _route(X, gW1, gb1, gW2, gb2):
    """Top-2 routing computed in float64 on the host."""
    g = np.maximum(X.astype(np.float64) @ gW1.astype(np.float64)
                   + gb1.astype(np.float64), 0.0)
    logits = g @ gW2.astype(np.float64) + gb2.astype(np.float64)   # [B, E]
    top2 = np.argpartition(-logits, 1, axis=1)[:, :2]              # [B, 2]
    l2 = np.take_along_axis(logits, top2, axis=1)
    ew = np.exp(l2 - l2.max(axis=1, keepdims=True))
    wts = ew / ew.sum(axis=1, keepdims=True)                       # [B, 2]
    return top2, wts.astype(np.float32)


def kernel(id_emb, llm_emb, W1, b1, W2, b2, gW1, gb1, gW2, gb2):
    global LAST_RESULT
    from concourse.bass_utils import run_bass_kernel_spmd

    X = np.concatenate([np.asarray(id_emb, np.float32),
                        np.asarray(llm_emb, np.float32)], axis=1)  # [B, IN]
    W1 = np.asarray(W1, np.float32); b1 = np.asarray(b1, np.float32)
    W2 = np.asarray(W2, np.float32); b2 = np.asarray(b2, np.float32)

    top2, wts = _route(X, np.asarray(gW1), np.asarray(gb1),
                       np.asarray(gW2), np.asarray(gb2))

    # ---- group (token, expert) pairs into blk-token blocks per expert ----
    per_e = []
    for e in range(E):
        mask = (top2 == e)                # [B, 2]
        ids = np.nonzero(mask.any(axis=1))[0]
        w_e = wts[mask]                   # row-major -> token order
        per_e.append((ids, w_e))
    counts = [len(ids) for ids, _ in per_e]

    # pick blk minimizing the critical path  ceil(nblocks/8) * blk
    best = None
    for cand in range(384, 513, 4):
        nb = sum(-(-c // cand) for c in counts if c)
        s_cand = max(1, -(-nb // NCORES))
        crit = s_cand * cand
        if s_cand <= 12 and (best is None or crit < best[0]):
            best = (crit, cand, s_cand)
    _, blk, S = best
    force = os.environ.get("KERNEL_FORCE_BLK")
    if force:
        blk = int(force)
        nb = sum(-(-c // blk) for c in counts if c)
        S = max(1, -(-nb // NCORES))

    blocks = []                           # (expert, ids, w)
    for e in range(E):
        ids, w_e = per_e[e]
        for i in range(0, len(ids), blk):
            blocks.append((e, ids[i:i + blk], w_e[i:i + blk]))
    assert len(blocks) <= NCORES * S

    # ---- per-expert device-layout weight packs (built once, fp16) ----
    used = sorted({e for e, _, _ in blocks})
    w1p, w2p, b1p = {}, {}, {}
    for e in used:
        w1p[e] = np.ascontiguousarray(
            W1[e].reshape(KT1, 128, HT, 128).transpose(2, 1, 0, 3)
        ).reshape(HT, 128, KT1 * 128).astype(np.float16)
        w2p[e] = np.ascontiguousarray(
            W2[e].reshape(KT2, 128, OT, 128).transpose(2, 1, 0, 3)
        ).reshape(OT, 128, KT2 * 128).astype(np.float16)
        b1p[e] = np.ascontiguousarray(b1[e].reshape(HT, 128).T)

    zero_xt = np.zeros((128, KT1 * blk), np.float16)
    e0 = used[0]

    # ---- per-core input maps ----
    in_maps = [dict() for _ in range(NCORES)]
    for bi, (e, ids, w) in enumerate(blocks):
        c, s = bi % NCORES, bi // NCORES
        n = len(ids)
        xb = np.zeros((blk, IN_DIM), np.float32)
        xb[:n] = X[ids]
        xt = np.ascontiguousarray(
            xb.T.reshape(KT1, 128, blk).transpose(1, 0, 2)
        ).reshape(128, KT1 * blk).astype(np.float16)
        m = in_maps[c]
        m[f"xt_{s}"] = xt
        m[f"w1_{s}"] = w1p[e]
        m[f"w2_{s}"] = w2p[e]
        m[f"b1_{s}"] = b1p[e]
    for c in range(NCORES):               # dummy slots
        m = in_maps[c]
        for s in range(S):
            if f"xt_{s}" not in m:
                m[f"xt_{s}"] = zero_xt
                m[f"w1_{s}"] = w1p[e0]
                m[f"w2_{s}"] = w2p[e0]
                m[f"b1_{s}"] = b1p[e0]

    # ---- run on the 8 cores ----
    nc = _get_nc(blk, S)
    trace = bool(int(os.environ.get("KERNEL_TRACE", "0")))
    res = run_bass_kernel_spmd(nc, in_maps, list(range(NCORES)), trace=trace)
    LAST_RESULT = res

    # ---- combine:  out[t] += w * (y + b2[e])  in expert order ----
    out = np.zeros((B, OUT_DIM), np.float32)
    for bi, (e, ids, w) in enumerate(blocks):
        c, s = bi % NCORES, bi // NCORES
        yt = np.asarray(res.results[c][f"yt_{s}"])        # [OT, 128, blk]
        y = yt.transpose(2, 0, 1).reshape(blk, OUT_DIM)[:len(ids)]
        out[ids] += w[:, None] * (y + b2[e][None, :])
    return out



# revision 2
# speedup vs baseline: 1.1464x; 1.1464x over previous
"""MoE adapter kernel for 8 Trainium2 NeuronCores.

Math (faithful to the reference): every token routes to its top-2 of 8
experts (gate = 2-layer MLP on the concat embedding); the output is the
softmax-weighted sum of the two selected experts' MLP outputs.  The
reference computes ALL experts densely with combine weights that are
exactly zero for unselected experts, so sparse top-2 computation is
mathematically identical (4x fewer FLOPs).

Strategy:
  - Host: gate + top-2 routing in float64, group (token, expert) pairs
    by expert.
  - Exact-fill packing: 32 bins of 512 tokens (4 slots x 8 cores) take
    every expert's full 512-blocks; the 2 spare bins take the two
    largest remainders whole; the remaining per-expert remainders are
    split into <=8 chunks of B = minimal feasible width (74 for the
    reference gate) and ride slot 0 as a SECOND SEGMENT, interleaved
    with the main 512-block at k-tile granularity (so the 2x LDWEIGHTS
    cost hides under the 512+B-cycle moving stream, and the B weight
    stream shares slot 0's DMA window).  Per-core streamed columns:
    2048 + B = 2122 vs 2200 for the uniform-block baseline.
  - Device (SPMD, same program on all 8 cores; per-core weights/tokens
    arrive as input data): per block, a 2-layer MLP
    [N,5120]x[5120,4096] -> relu -> x[4096,2048], fp16 operands with
    fp32 PSUM accumulation, weights stationary / activations moving.
    ~20 dummy warm-up matmuls at kernel start trip the PE HAM clock
    gate to 2.4 GHz during the otherwise-idle DMA ramp.  Main stream
    DMAs ride the sync HWDGE ring, the B-segment stream rides the
    scalar ring.
  - Host: scatter-add  w * (y + b2)  into the [8192, 2048] output.
"""

import os
import numpy as np

B_TOK = 8192
IN_DIM = 5120
HID = 4096
OUT_DIM = 2048
E = 8
NCORES = 8
KT1 = IN_DIM // 128          # 40 k-tiles, layer 1
HT = HID // 128              # 32 hid tiles
KT2 = HID // 128             # 32 k-tiles, layer 2
OT = OUT_DIM // 128          # 16 out tiles
ABLK = 512                   # main block size
NSLOT = 4                    # 4 slots of 512 (slot 3 also carries B-seg)

LAST_RESULT = None


def _build_bass(BSEG):
    """4 slots of 512 columns; slot 3 additionally carries a BSEG-column
    second segment (different expert) interleaved at k-tile granularity."""
    import concourse.bass as bass
    import concourse.mybir as mybir
    import concourse.tile as tile
    from concourse import bacc
    from concourse.bass import ts

    f16 = mybir.dt.float16
    f32 = mybir.dt.float32

    nc = bacc.Bacc("TRN2", target_bir_lowering=False, debug=False,
                   num_devices=NCORES)

    xt_d, w1_d, w2_d, b1_d, yt_d = [], [], [], [], []
    for s in range(NSLOT):
        xt_d.append(nc.dram_tensor(f"xt_{s}", [128, KT1 * ABLK], f16,
                                   kind="ExternalInput"))
        w1_d.append(nc.dram_tensor(f"w1_{s}", [HT, 128, KT1 * 128], f16,
                                   kind="ExternalInput"))
        w2_d.append(nc.dram_tensor(f"w2_{s}", [OT, 128, KT2 * 128], f16,
                                   kind="ExternalInput"))
        b1_d.append(nc.dram_tensor(f"b1_{s}", [128, HT], f32,
                                   kind="ExternalInput"))
        yt_d.append(nc.dram_tensor(f"yt_{s}", [OT, 128, ABLK], f32,
                                   kind="ExternalOutput"))
    if BSEG:
        xtb_d = nc.dram_tensor("xtb", [128, KT1 * BSEG], f16,
                               kind="ExternalInput")
        w1b_d = nc.dram_tensor("w1b", [HT, 128, KT1 * 128], f16,
                               kind="ExternalInput")
        w2b_d = nc.dram_tensor("w2b", [OT, 128, KT2 * 128], f16,
                               kind="ExternalInput")
        b1b_d = nc.dram_tensor("b1b", [128, HT], f32, kind="ExternalInput")
        ytb_d = nc.dram_tensor("ytb", [OT, 128, BSEG], f32,
                               kind="ExternalOutput")

    relu = mybir.ActivationFunctionType.Relu

    with tile.TileContext(nc) as tc:
        with (
            tc.tile_pool(name="xt", bufs=2) as xt_pool,
            tc.tile_pool(name="xtb", bufs=1) as xtb_pool,
            tc.tile_pool(name="w1", bufs=2) as w1_pool,
            tc.tile_pool(name="w1b", bufs=2) as w1b_pool,
            tc.tile_pool(name="w2", bufs=2) as w2_pool,
            tc.tile_pool(name="w2b", bufs=2) as w2b_pool,
            tc.tile_pool(name="h", bufs=1) as h_pool,
            tc.tile_pool(name="hb", bufs=1) as hb_pool,
            tc.tile_pool(name="b", bufs=2) as b_pool,
            tc.tile_pool(name="y", bufs=2) as y_pool,
            tc.tile_pool(name="yb", bufs=2) as yb_pool,
            tc.tile_pool(name="ps1", bufs=2, space="PSUM") as ps1_pool,
            tc.tile_pool(name="ps2", bufs=2, space="PSUM") as ps2_pool,
            tc.tile_pool(name="ps1b", bufs=2, space="PSUM") as ps1b_pool,
            tc.tile_pool(name="ps2b", bufs=2, space="PSUM") as ps2b_pool,
            tc.tile_pool(name="warm", bufs=1) as warm_pool,
        ):
            # HAM warmup: ~20 dummy matmuls issued with no input
            # dependencies fill the otherwise-idle ~16us startup
            # (preamble + first DMA latency) with PE activity, so the
            # clock gate is at 8/8 (2.4 GHz) before the first real
            # matmul instead of ~20us into the run.
            wsrc = warm_pool.tile([128, 512], f16, tag="warm")
            nc.vector.memset(wsrc[:], 0.0)
            wps = ps1_pool.tile([128, ABLK], f32, tag="ps1")
            for i in range(20):
                nc.tensor.matmul(wps[:], wsrc[:, :128], wsrc[:],
                                 start=(i == 0), stop=(i == 19))

            for s in range(NSLOT):
                two = BSEG and s == 0

                xt = xt_pool.tile([128, KT1 * ABLK], f16, tag="xt")
                b1t = b_pool.tile([128, HT], f32, tag="b1")
                nc.sync.dma_start(out=b1t[:], in_=b1_d[s].ap())
                if two:
                    xtb = xtb_pool.tile([128, KT1 * BSEG], f16, tag="xtb")
                    b1tb = b_pool.tile([128, HT], f32, tag="b1b")
                    nc.scalar.dma_start(out=b1tb[:], in_=b1b_d.ap())
                    hb_sb = hb_pool.tile([128, HT * BSEG], f16, tag="hb")

                h_sb = h_pool.tile([128, HT * ABLK], f16, tag="h")

                xt_dma = {k0: 10 for k0 in range(0, KT1, 10)}

                for h in range(HT):
                    w1t = w1_pool.tile([128, KT1 * 128], f16, tag="w1")
                    nc.sync.dma_start(out=w1t[:], in_=w1_d[s].ap()[h])
                    if two:
                        w1tb = w1b_pool.tile([128, KT1 * 128], f16, tag="w1b")
                        nc.scalar.dma_start(out=w1tb[:], in_=w1b_d.ap()[h])

                    ps = ps1_pool.tile([128, ABLK], f32, tag="ps1")
                    if two:
                        psb = ps1b_pool.tile([128, 512], f32, tag="ps1b")
                    for k in range(KT1):
                        if h == 0 and k in xt_dma:
                            k0, kn = k, xt_dma[k]
                            cols = slice(k0 * ABLK, (k0 + kn) * ABLK)
                            nc.sync.dma_start(out=xt[:, cols],
                                              in_=xt_d[s].ap()[:, cols])
                            if two:
                                colsb = slice(k0 * BSEG, (k0 + kn) * BSEG)
                                nc.scalar.dma_start(out=xtb[:, colsb],
                                                    in_=xtb_d.ap()[:, colsb])
                        nc.tensor.matmul(ps[:], w1t[:, ts(k, 128)],
                                         xt[:, ts(k, ABLK)],
                                         start=(k == 0), stop=(k == KT1 - 1))
                        if two:
                            nc.tensor.matmul(psb[:, :BSEG],
                                             w1tb[:, ts(k, 128)],
                                             xtb[:, ts(k, BSEG)],
                                             start=(k == 0),
                                             stop=(k == KT1 - 1))
                    nc.scalar.activation(h_sb[:, ts(h, ABLK)], ps[:], relu,
                                         bias=b1t[:, h:h + 1])
                    if two:
                        nc.scalar.activation(hb_sb[:, ts(h, BSEG)],
                                             psb[:, :BSEG], relu,
                                             bias=b1tb[:, h:h + 1])

                for o in range(OT):
                    w2t = w2_pool.tile([128, KT2 * 128], f16, tag="w2")
                    nc.sync.dma_start(out=w2t[:], in_=w2_d[s].ap()[o])
                    if two:
                        w2tb = w2b_pool.tile([128, KT2 * 128], f16, tag="w2b")
                        nc.scalar.dma_start(out=w2tb[:], in_=w2b_d.ap()[o])
                    ps2 = ps2_pool.tile([128, ABLK], f32, tag="ps2")
                    if two:
                        ps2b = ps2b_pool.tile([128, 512], f32, tag="ps2b")
                    for k in range(KT2):
                        nc.tensor.matmul(ps2[:], w2t[:, ts(k, 128)],
                                         h_sb[:, ts(k, ABLK)],
                                         start=(k == 0), stop=(k == KT2 - 1))
                        if two:
                            nc.tensor.matmul(ps2b[:, :BSEG],
                                             w2tb[:, ts(k, 128)],
                                             hb_sb[:, ts(k, BSEG)],
                                             start=(k == 0),
                                             stop=(k == KT2 - 1))
                    yt_sb = y_pool.tile([128, ABLK], f32, tag="y")
                    nc.vector.tensor_copy(yt_sb[:], ps2[:])
                    nc.sync.dma_start(out=yt_d[s].ap()[o], in_=yt_sb[:])
                    if two:
                        ytb_sb = yb_pool.tile([128, BSEG], f32, tag="yb")
                        nc.vector.tensor_copy(ytb_sb[:], ps2b[:, :BSEG])
                        nc.sync.dma_start(out=ytb_d.ap()[o], in_=ytb_sb[:])

    nc.compile()
    return nc


_NC = {}


def _get_nc(bseg):
    if bseg not in _NC:
        _NC[bseg] = _build_bass(bseg)
    return _NC[bseg]


def _route(X, gW1, gb1, gW2, gb2):
    """Top-2 routing computed in float64 on the host."""
    g = np.maximum(X.astype(np.float64) @ gW1.astype(np.float64)
                   + gb1.astype(np.float64), 0.0)
    logits = g @ gW2.astype(np.float64) + gb2.astype(np.float64)   # [B, E]
    top2 = np.argpartition(-logits, 1, axis=1)[:, :2]              # [B, 2]
    l2 = np.take_along_axis(logits, top2, axis=1)
    ew = np.exp(l2 - l2.max(axis=1, keepdims=True))
    wts = ew / ew.sum(axis=1, keepdims=True)                       # [B, 2]
    return top2, wts.astype(np.float32)


def _pack_xt(xb, blk):
    """[blk, IN_DIM] fp32 -> [128, KT1*blk] fp16 device layout."""
    return np.ascontiguousarray(
        xb.T.reshape(KT1, 128, blk).transpose(1, 0, 2)
    ).reshape(128, KT1 * blk).astype(np.float16)


def kernel(id_emb, llm_emb, W1, b1, W2, b2, gW1, gb1, gW2, gb2):
    global LAST_RESULT
    from concourse.bass_utils import run_bass_kernel_spmd

    X = np.concatenate([np.asarray(id_emb, np.float32),
                        np.asarray(llm_emb, np.float32)], axis=1)  # [B, IN]
    W1 = np.asarray(W1, np.float32); b1 = np.asarray(b1, np.float32)
    W2 = np.asarray(W2, np.float32); b2 = np.asarray(b2, np.float32)

    top2, wts = _route(X, np.asarray(gW1), np.asarray(gb1),
                       np.asarray(gW2), np.asarray(gb2))

    # ---- group (token, expert) pairs ----
    per_e = []
    for e in range(E):
        mask = (top2 == e)                # [B, 2]
        ids = np.nonzero(mask.any(axis=1))[0]
        w_e = wts[mask]                   # row-major -> token order
        per_e.append((ids, w_e))

    # ---- exact-fill packing: 32 bins of 512 + up to 8 B-bins ----
    full_bins = []                        # (expert, ids, w) with len<=512
    spills = []
    for e in range(E):
        ids, w_e = per_e[e]
        n = len(ids)
        nf = n // ABLK
        for i in range(nf):
            sl = slice(i * ABLK, (i + 1) * ABLK)
            full_bins.append((e, ids[sl], w_e[sl]))
        if n % ABLK:
            sl = slice(nf * ABLK, n)
            spills.append((n - nf * ABLK, e, ids[sl], w_e[sl]))
    spills.sort(key=lambda t: -t[0])
    spares = NSLOT * NCORES - len(full_bins)
    assert spares >= 0
    k = min(spares, len(spills))
    for sz, e, ids, w_e in spills[:k]:
        full_bins.append((e, ids, w_e))
    rest = spills[k:]
    # minimize the B-segment width: split remaining spills into <=8
    # single-expert chunks of width B, choosing the smallest feasible B
    if rest:
        lo, hi = 1, rest[0][0]
        while lo < hi:
            mid = (lo + hi) // 2
            if sum(-(-sz // mid) for sz, _, _, _ in rest) <= NCORES:
                hi = mid
            else:
                lo = mid + 1
        BSEG = lo
        split = []
        for sz, e, ids, w_e in rest:
            for i in range(0, sz, BSEG):
                j = min(i + BSEG, sz)
                split.append((j - i, e, ids[i:j], w_e[i:j]))
        rest = split
    assert len(rest) <= NCORES, "more remainder bins than cores"
    BSEG = rest[0][0] if rest else 0
    BSEG = max((sz for sz, _, _, _ in rest), default=0)
    while len(full_bins) < NSLOT * NCORES:
        full_bins.append(None)

    # ---- per-expert device-layout weight packs (fp16) ----
    used = sorted({t[0] for t in full_bins if t} | {t[1] for t in rest})
    w1p, w2p, b1p = {}, {}, {}
    for e in used:
        w1p[e] = np.ascontiguousarray(
            W1[e].reshape(KT1, 128, HT, 128).transpose(2, 1, 0, 3)
        ).reshape(HT, 128, KT1 * 128).astype(np.float16)
        w2p[e] = np.ascontiguousarray(
            W2[e].reshape(KT2, 128, OT, 128).transpose(2, 1, 0, 3)
        ).reshape(OT, 128, KT2 * 128).astype(np.float16)
        b1p[e] = np.ascontiguousarray(b1[e].reshape(HT, 128).T)
    e0 = used[0]

    # ---- per-core input maps ----
    in_maps = [dict() for _ in range(NCORES)]
    # A-bins: core c, slot s <- full_bins[s*8 + c]
    placed = []                           # (core, key, expert, ids, w, blk)
    for bi, binfo in enumerate(full_bins):
        c, s = bi % NCORES, bi // NCORES
        m = in_maps[c]
        if binfo is None:
            m[f"xt_{s}"] = np.zeros((128, KT1 * ABLK), np.float16)
            m[f"w1_{s}"] = w1p[e0]
            m[f"w2_{s}"] = w2p[e0]
            m[f"b1_{s}"] = b1p[e0]
            continue
        e, ids, w_e = binfo
        n = len(ids)
        xb = np.zeros((ABLK, IN_DIM), np.float32)
        xb[:n] = X[ids]
        m[f"xt_{s}"] = _pack_xt(xb, ABLK)
        m[f"w1_{s}"] = w1p[e]
        m[f"w2_{s}"] = w2p[e]
        m[f"b1_{s}"] = b1p[e]
        placed.append((c, f"yt_{s}", e, ids, w_e))
    if BSEG:
        for c in range(NCORES):
            m = in_maps[c]
            if c < len(rest):  # B-segment rides slot 0
                sz, e, ids, w_e = rest[c]
                xb = np.zeros((BSEG, IN_DIM), np.float32)
                xb[:sz] = X[ids]
                m["xtb"] = _pack_xt(xb, BSEG)
                m["w1b"] = w1p[e]
                m["w2b"] = w2p[e]
                m["b1b"] = b1p[e]
                placed.append((c, "ytb", e, ids, w_e))
            else:
                m["xtb"] = np.zeros((128, KT1 * BSEG), np.float16)
                m["w1b"] = w1p[e0]
                m["w2b"] = w2p[e0]
                m["b1b"] = b1p[e0]

    # ---- run on the 8 cores ----
    nc = _get_nc(BSEG)
    trace = bool(int(os.environ.get("KERNEL_TRACE", "0")))
    res = run_bass_kernel_spmd(nc, in_maps, list(range(NCORES)), trace=trace)
    LAST_RESULT = res

    # ---- combine:  out[t] += w * (y + b2[e]) ----
    out = np.zeros((B_TOK, OUT_DIM), np.float32)
    for c, key, e, ids, w_e in placed:
        yt = np.asarray(res.results[c][key])          # [OT, 128, blk]
        blk = yt.shape[2]
        y = yt.transpose(2, 0, 1).reshape(blk, OUT_DIM)[:len(ids)]
        out[ids] += w_e[:, None] * (y + b2[e][None, :])
    return out


# revision 3
# speedup vs baseline: 1.1491x; 1.0024x over previous
"""MoE adapter kernel for 8 Trainium2 NeuronCores.

Math (faithful to the reference): every token routes to its top-2 of 8
experts (gate = 2-layer MLP on the concat embedding); the output is the
softmax-weighted sum of the two selected experts' MLP outputs.  The
reference computes ALL experts densely with combine weights that are
exactly zero for unselected experts, so sparse top-2 computation is
mathematically identical (4x fewer FLOPs).

Strategy:
  - Host: gate + top-2 routing in float64, group (token, expert) pairs
    by expert.
  - Exact-fill packing: 32 bins of 512 tokens (4 slots x 8 cores) take
    every expert's full 512-blocks; the 2 spare bins take the two
    largest remainders whole; the remaining per-expert remainders are
    split into <=8 chunks of B = minimal feasible width (74 for the
    reference gate) and ride slot 0 as a SECOND SEGMENT, interleaved
    with the main 512-block at k-tile granularity (so the 2x LDWEIGHTS
    cost hides under the 512+B-cycle moving stream, and the B weight
    stream shares slot 0's DMA window).  Per-core streamed columns:
    2048 + B = 2122 vs 2200 for the uniform-block baseline.
  - Device (SPMD, same program on all 8 cores; per-core weights/tokens
    arrive as input data): per block, a 2-layer MLP
    [N,5120]x[5120,4096] -> relu -> x[4096,2048], fp16 operands with
    fp32 PSUM accumulation, weights stationary / activations moving.
    ~20 dummy warm-up matmuls at kernel start trip the PE HAM clock
    gate to 2.4 GHz during the otherwise-idle DMA ramp.  Main stream
    DMAs ride the sync HWDGE ring, the B-segment stream rides the
    scalar ring.
  - Host: scatter-add  w * (y + b2)  into the [8192, 2048] output.
"""

import os
import numpy as np

B_TOK = 8192
IN_DIM = 5120
HID = 4096
OUT_DIM = 2048
E = 8
NCORES = 8
KT1 = IN_DIM // 128          # 40 k-tiles, layer 1
HT = HID // 128              # 32 hid tiles
KT2 = HID // 128             # 32 k-tiles, layer 2
OT = OUT_DIM // 128          # 16 out tiles
ABLK = 512                   # main block size
NSLOT = 4                    # 4 slots of 512 (slot 3 also carries B-seg)

LAST_RESULT = None


def _build_bass(BSEG):
    """4 slots of 512 columns; slot 3 additionally carries a BSEG-column
    second segment (different expert) interleaved at k-tile granularity."""
    import concourse.bass as bass
    import concourse.mybir as mybir
    import concourse.tile as tile
    from concourse import bacc
    from concourse.bass import ts

    f16 = mybir.dt.float16
    f32 = mybir.dt.float32

    nc = bacc.Bacc("TRN2", target_bir_lowering=False, debug=False,
                   num_devices=NCORES)

    xt_d, w1_d, w2_d, b1_d, yt_d = [], [], [], [], []
    for s in range(NSLOT):
        xt_d.append(nc.dram_tensor(f"xt_{s}", [128, KT1 * ABLK], f16,
                                   kind="ExternalInput"))
        w1_d.append(nc.dram_tensor(f"w1_{s}", [HT, 128, KT1 * 128], f16,
                                   kind="ExternalInput"))
        w2_d.append(nc.dram_tensor(f"w2_{s}", [OT, 128, KT2 * 128], f16,
                                   kind="ExternalInput"))
        b1_d.append(nc.dram_tensor(f"b1_{s}", [128, HT], f32,
                                   kind="ExternalInput"))
        yt_d.append(nc.dram_tensor(f"yt_{s}", [OT, 128, ABLK], f32,
                                   kind="ExternalOutput"))
    if BSEG:
        xtb_d = nc.dram_tensor("xtb", [128, KT1 * BSEG], f16,
                               kind="ExternalInput")
        w1b_d = nc.dram_tensor("w1b", [HT, 128, KT1 * 128], f16,
                               kind="ExternalInput")
        w2b_d = nc.dram_tensor("w2b", [OT, 128, KT2 * 128], f16,
                               kind="ExternalInput")
        b1b_d = nc.dram_tensor("b1b", [128, HT], f32, kind="ExternalInput")
        ytb_d = nc.dram_tensor("ytb", [OT, 128, BSEG], f32,
                               kind="ExternalOutput")

    relu = mybir.ActivationFunctionType.Relu

    with tile.TileContext(nc) as tc:
        with (
            tc.tile_pool(name="xt", bufs=2) as xt_pool,
            tc.tile_pool(name="xtb", bufs=1) as xtb_pool,
            tc.tile_pool(name="w1", bufs=2) as w1_pool,
            tc.tile_pool(name="w1b", bufs=2) as w1b_pool,
            tc.tile_pool(name="w2", bufs=2) as w2_pool,
            tc.tile_pool(name="w2b", bufs=2) as w2b_pool,
            tc.tile_pool(name="h", bufs=1) as h_pool,
            tc.tile_pool(name="hb", bufs=1) as hb_pool,
            tc.tile_pool(name="b", bufs=2) as b_pool,
            tc.tile_pool(name="y", bufs=2) as y_pool,
            tc.tile_pool(name="yb", bufs=2) as yb_pool,
            tc.tile_pool(name="ps1", bufs=2, space="PSUM") as ps1_pool,
            tc.tile_pool(name="ps2", bufs=2, space="PSUM") as ps2_pool,
            tc.tile_pool(name="ps1b", bufs=2, space="PSUM") as ps1b_pool,
            tc.tile_pool(name="ps2b", bufs=2, space="PSUM") as ps2b_pool,
            tc.tile_pool(name="warm", bufs=1) as warm_pool,
        ):
            # HAM warmup: ~20 dummy matmuls issued with no input
            # dependencies fill the otherwise-idle ~16us startup
            # (preamble + first DMA latency) with PE activity, so the
            # clock gate is at 8/8 (2.4 GHz) before the first real
            # matmul instead of ~20us into the run.
            wsrc = warm_pool.tile([128, 512], f16, tag="warm")
            nc.vector.memset(wsrc[:], 0.0)
            wps = ps1_pool.tile([128, ABLK], f32, tag="ps1")
            for i in range(20):
                nc.tensor.matmul(wps[:], wsrc[:, :128], wsrc[:],
                                 start=(i == 0), stop=(i == 19))

            for s in range(NSLOT):
                two = BSEG and s == 0

                xt = xt_pool.tile([128, KT1 * ABLK], f16, tag="xt")
                b1t = b_pool.tile([128, HT], f32, tag="b1")
                nc.sync.dma_start(out=b1t[:], in_=b1_d[s].ap())
                if two:
                    xtb = xtb_pool.tile([128, KT1 * BSEG], f16, tag="xtb")
                    b1tb = b_pool.tile([128, HT], f32, tag="b1b")
                    nc.scalar.dma_start(out=b1tb[:], in_=b1b_d.ap())
                    hb_sb = hb_pool.tile([128, HT * BSEG], f16, tag="hb")

                h_sb = h_pool.tile([128, HT * ABLK], f16, tag="h")

                xt_dma = {k0: 10 for k0 in range(0, KT1, 10)}

                for h in range(HT):
                    w1t = w1_pool.tile([128, KT1 * 128], f16, tag="w1")
                    nc.sync.dma_start(out=w1t[:], in_=w1_d[s].ap()[h])
                    if two:
                        w1tb = w1b_pool.tile([128, KT1 * 128], f16, tag="w1b")
                        nc.scalar.dma_start(out=w1tb[:], in_=w1b_d.ap()[h])

                    ps = ps1_pool.tile([128, ABLK], f32, tag="ps1")
                    if two:
                        psb = ps1b_pool.tile([128, 512], f32, tag="ps1b")
                    for k in range(KT1):
                        if h == 0 and k in xt_dma:
                            k0, kn = k, xt_dma[k]
                            cols = slice(k0 * ABLK, (k0 + kn) * ABLK)
                            # slot 0: sync ring (startup-critical, serial
                            # with nothing). slots 1-3: scalar ring, which
                            # idles during the previous slot's L2 -> the
                            # xt lands before the slot boundary instead of
                            # queueing behind the w2 stream on sync.
                            eng = nc.sync if s == 0 else nc.scalar
                            eng.dma_start(out=xt[:, cols],
                                          in_=xt_d[s].ap()[:, cols])
                            if two:
                                colsb = slice(k0 * BSEG, (k0 + kn) * BSEG)
                                nc.scalar.dma_start(out=xtb[:, colsb],
                                                    in_=xtb_d.ap()[:, colsb])
                        nc.tensor.matmul(ps[:], w1t[:, ts(k, 128)],
                                         xt[:, ts(k, ABLK)],
                                         start=(k == 0), stop=(k == KT1 - 1))
                        if two:
                            nc.tensor.matmul(psb[:, :BSEG],
                                             w1tb[:, ts(k, 128)],
                                             xtb[:, ts(k, BSEG)],
                                             start=(k == 0),
                                             stop=(k == KT1 - 1))
                    nc.scalar.activation(h_sb[:, ts(h, ABLK)], ps[:], relu,
                                         bias=b1t[:, h:h + 1])
                    if two:
                        nc.scalar.activation(hb_sb[:, ts(h, BSEG)],
                                             psb[:, :BSEG], relu,
                                             bias=b1tb[:, h:h + 1])

                for o in range(OT):
                    w2t = w2_pool.tile([128, KT2 * 128], f16, tag="w2")
                    nc.sync.dma_start(out=w2t[:], in_=w2_d[s].ap()[o])
                    if two:
                        w2tb = w2b_pool.tile([128, KT2 * 128], f16, tag="w2b")
                        nc.scalar.dma_start(out=w2tb[:], in_=w2b_d.ap()[o])
                    ps2 = ps2_pool.tile([128, ABLK], f32, tag="ps2")
                    if two:
                        ps2b = ps2b_pool.tile([128, 512], f32, tag="ps2b")
                    for k in range(KT2):
                        nc.tensor.matmul(ps2[:], w2t[:, ts(k, 128)],
                                         h_sb[:, ts(k, ABLK)],
                                         start=(k == 0), stop=(k == KT2 - 1))
                        if two:
                            nc.tensor.matmul(ps2b[:, :BSEG],
                                             w2tb[:, ts(k, 128)],
                                             hb_sb[:, ts(k, BSEG)],
                                             start=(k == 0),
                                             stop=(k == KT2 - 1))
                    yt_sb = y_pool.tile([128, ABLK], f32, tag="y")
                    nc.vector.tensor_copy(yt_sb[:], ps2[:])
                    nc.sync.dma_start(out=yt_d[s].ap()[o], in_=yt_sb[:])
                    if two:
                        ytb_sb = yb_pool.tile([128, BSEG], f32, tag="yb")
                        nc.vector.tensor_copy(ytb_sb[:], ps2b[:, :BSEG])
                        nc.sync.dma_start(out=ytb_d.ap()[o], in_=ytb_sb[:])

    nc.compile()
    return nc


_NC = {}


def _get_nc(bseg):
    if bseg not in _NC:
        _NC[bseg] = _build_bass(bseg)
    return _NC[bseg]


def _route(X, gW1, gb1, gW2, gb2):
    """Top-2 routing computed in float64 on the host."""
    g = np.maximum(X.astype(np.float64) @ gW1.astype(np.float64)
                   + gb1.astype(np.float64), 0.0)
    logits = g @ gW2.astype(np.float64) + gb2.astype(np.float64)   # [B, E]
    top2 = np.argpartition(-logits, 1, axis=1)[:, :2]              # [B, 2]
    l2 = np.take_along_axis(logits, top2, axis=1)
    ew = np.exp(l2 - l2.max(axis=1, keepdims=True))
    wts = ew / ew.sum(axis=1, keepdims=True)                       # [B, 2]
    return top2, wts.astype(np.float32)


def _pack_xt(xb, blk):
    """[blk, IN_DIM] fp32 -> [128, KT1*blk] fp16 device layout."""
    return np.ascontiguousarray(
        xb.T.reshape(KT1, 128, blk).transpose(1, 0, 2)
    ).reshape(128, KT1 * blk).astype(np.float16)


def kernel(id_emb, llm_emb, W1, b1, W2, b2, gW1, gb1, gW2, gb2):
    global LAST_RESULT
    from concourse.bass_utils import run_bass_kernel_spmd

    X = np.concatenate([np.asarray(id_emb, np.float32),
                        np.asarray(llm_emb, np.float32)], axis=1)  # [B, IN]
    W1 = np.asarray(W1, np.float32); b1 = np.asarray(b1, np.float32)
    W2 = np.asarray(W2, np.float32); b2 = np.asarray(b2, np.float32)

    top2, wts = _route(X, np.asarray(gW1), np.asarray(gb1),
                       np.asarray(gW2), np.asarray(gb2))

    # ---- group (token, expert) pairs ----
    per_e = []
    for e in range(E):
        mask = (top2 == e)                # [B, 2]
        ids = np.nonzero(mask.any(axis=1))[0]
        w_e = wts[mask]                   # row-major -> token order
        per_e.append((ids, w_e))

    # ---- exact-fill packing: 32 bins of 512 + up to 8 B-bins ----
    full_bins = []                        # (expert, ids, w) with len<=512
    spills = []
    for e in range(E):
        ids, w_e = per_e[e]
        n = len(ids)
        nf = n // ABLK
        for i in range(nf):
            sl = slice(i * ABLK, (i + 1) * ABLK)
            full_bins.append((e, ids[sl], w_e[sl]))
        if n % ABLK:
            sl = slice(nf * ABLK, n)
            spills.append((n - nf * ABLK, e, ids[sl], w_e[sl]))
    spills.sort(key=lambda t: -t[0])
    spares = NSLOT * NCORES - len(full_bins)
    assert spares >= 0
    k = min(spares, len(spills))
    for sz, e, ids, w_e in spills[:k]:
        full_bins.append((e, ids, w_e))
    rest = spills[k:]
    # minimize the B-segment width: split remaining spills into <=8
    # single-expert chunks of width B, choosing the smallest feasible B
    if rest:
        lo, hi = 1, rest[0][0]
        while lo < hi:
            mid = (lo + hi) // 2
            if sum(-(-sz // mid) for sz, _, _, _ in rest) <= NCORES:
                hi = mid
            else:
                lo = mid + 1
        BSEG = lo
        split = []
        for sz, e, ids, w_e in rest:
            for i in range(0, sz, BSEG):
                j = min(i + BSEG, sz)
                split.append((j - i, e, ids[i:j], w_e[i:j]))
        rest = split
    assert len(rest) <= NCORES, "more remainder bins than cores"
    BSEG = rest[0][0] if rest else 0
    BSEG = max((sz for sz, _, _, _ in rest), default=0)
    while len(full_bins) < NSLOT * NCORES:
        full_bins.append(None)

    # ---- per-expert device-layout weight packs (fp16) ----
    used = sorted({t[0] for t in full_bins if t} | {t[1] for t in rest})
    w1p, w2p, b1p = {}, {}, {}
    for e in used:
        w1p[e] = np.ascontiguousarray(
            W1[e].reshape(KT1, 128, HT, 128).transpose(2, 1, 0, 3)
        ).reshape(HT, 128, KT1 * 128).astype(np.float16)
        w2p[e] = np.ascontiguousarray(
            W2[e].reshape(KT2, 128, OT, 128).transpose(2, 1, 0, 3)
        ).reshape(OT, 128, KT2 * 128).astype(np.float16)
        b1p[e] = np.ascontiguousarray(b1[e].reshape(HT, 128).T)
    e0 = used[0]

    # ---- per-core input maps ----
    in_maps = [dict() for _ in range(NCORES)]
    # A-bins: core c, slot s <- full_bins[s*8 + c]
    placed = []                           # (core, key, expert, ids, w, blk)
    for bi, binfo in enumerate(full_bins):
        c, s = bi % NCORES, bi // NCORES
        m = in_maps[c]
        if binfo is None:
            m[f"xt_{s}"] = np.zeros((128, KT1 * ABLK), np.float16)
            m[f"w1_{s}"] = w1p[e0]
            m[f"w2_{s}"] = w2p[e0]
            m[f"b1_{s}"] = b1p[e0]
            continue
        e, ids, w_e = binfo
        n = len(ids)
        xb = np.zeros((ABLK, IN_DIM), np.float32)
        xb[:n] = X[ids]
        m[f"xt_{s}"] = _pack_xt(xb, ABLK)
        m[f"w1_{s}"] = w1p[e]
        m[f"w2_{s}"] = w2p[e]
        m[f"b1_{s}"] = b1p[e]
        placed.append((c, f"yt_{s}", e, ids, w_e))
    if BSEG:
        for c in range(NCORES):
            m = in_maps[c]
            if c < len(rest):  # B-segment rides slot 0
                sz, e, ids, w_e = rest[c]
                xb = np.zeros((BSEG, IN_DIM), np.float32)
                xb[:sz] = X[ids]
                m["xtb"] = _pack_xt(xb, BSEG)
                m["w1b"] = w1p[e]
                m["w2b"] = w2p[e]
                m["b1b"] = b1p[e]
                placed.append((c, "ytb", e, ids, w_e))
            else:
                m["xtb"] = np.zeros((128, KT1 * BSEG), np.float16)
                m["w1b"] = w1p[e0]
                m["w2b"] = w2p[e0]
                m["b1b"] = b1p[e0]

    # ---- run on the 8 cores ----
    nc = _get_nc(BSEG)
    trace = bool(int(os.environ.get("KERNEL_TRACE", "0")))
    res = run_bass_kernel_spmd(nc, in_maps, list(range(NCORES)), trace=trace)
    LAST_RESULT = res

    # ---- combine:  out[t] += w * (y + b2[e]) ----
    out = np.zeros((B_TOK, OUT_DIM), np.float32)
    for c, key, e, ids, w_e in placed:
        yt = np.asarray(res.results[c][key])          # [OT, 128, blk]
        blk = yt.shape[2]
        y = yt.transpose(2, 0, 1).reshape(blk, OUT_DIM)[:len(ids)]
        out[ids] += w_e[:, None] * (y + b2[e][None, :])
    return out
